# revision 31
# baseline (speedup 1.0000x reference)
"""BiCEBertAttention TRN2 kernel (v2: bf16 datapath).

Reference semantics (B=2, T=2048, C=768, H=12 heads, D=64):
  qkv = x @ Wqkv_w.T + Wqkv_b ; heads 0-5 causal attention, heads 6-11
  anti-causal attention; out = ctx @ Wo_w.T + Wo_b.

Sharding: 8 cores = 2 batches x 4 head-groups (3 heads each). Head groups
0,1 are causal; groups 2,3 anti-causal. Anti-causal cores receive the
sequence REVERSED on the host, so all 8 cores run one identical causal
program (SPMD). Wqkv is column-sharded; Wo is row-sharded -> each core
returns a partial [T, C] output; the host sums the 4 partials per batch.

v2 changes vs baseline (fp32r):
  - all PE operands bf16 (HW fp32r matmul streams at ~0.72 col/ns vs
    ~0.52 for bf16; PSUM accumulation stays fp32, rel-err ~1e-3)
  - q and k interleaved in ONE [128, T] tile per head (single PSUM
    evacuation copy instead of two [64, .] copies)
  - v width 198 = 3 heads x (64 + ones-col + pad), no 256 padding
  - diagonal score pairs compact-packed: row0 at cols [off0:512], row1 at
    [512:1024-off1], so ONE exp instruction covers exactly the valid
    region (ACT per-inst overhead is ~600 ns; no garbage columns)
  - causal mask folded into the score accumulation on the PE: a second
    matmul adds strict-upper-tri x (-3744) via tri_neg^T @ [I|0], so exp
    underflows masked entries to 0 (no Pool mask multiplies, shorter
    exp->AV chain)
  - 1/8 softmax scale folded into the q rows of Wqkv host-side (exp
    runs with scale=1)
  - normalize: per-J batched reciprocal ([3,512] strip costs the same
    3.4us DVE as [1,512]) -> partition_broadcast on Pool (replaces the
    PE bc matmul) -> tensor_tensor into ctx. o_ps is released quickly
    by the den/tmp copies so the opool never stalls the PE.
"""

import numpy as np
import ml_dtypes

import concourse.bass as bass
import concourse.mybir as mybir
import concourse.tile as tile
from concourse import bacc
from concourse.bass_utils import run_bass_kernel_spmd
from concourse.masks import make_upper_triangular, make_identity

B, T, C, H, D = 2, 2048, 768, 12, 64
N_LEFT = 6
HPC = 3          # heads per core
NCORES = 8
KO = C // 128    # 6 contraction subtiles
NT = T // 128    # 16 key tiles
NJ = T // 512    # 4 query blocks
HS = 66          # per-head v stride: 64 dims + ones col + 1 pad
VW = HPC * HS    # 198
f32 = mybir.dt.float32
bf16 = mybir.dt.bfloat16
Exp = mybir.ActivationFunctionType.Exp
bft = ml_dtypes.bfloat16

_NC_CACHE: dict = {}


def build_nc(use_pad: bool, use_bqk: bool, reps: int = 1):
    nc = bacc.Bacc("TRN2", target_bir_lowering=False, debug=False)

    xT = nc.declare_dram_parameter("xT", [C, T], bf16, isOutput=False)
    wqk = nc.declare_dram_parameter("wqk", [C, HPC * 128], bf16, isOutput=False)
    wv = nc.declare_dram_parameter("wv", [C, VW], bf16, isOutput=False)
    bqk = nc.declare_dram_parameter("bqk", [1, HPC * 128], bf16, isOutput=False)
    bv = nc.declare_dram_parameter("bv", [1, VW], bf16, isOutput=False)
    wo = nc.declare_dram_parameter("wo", [HPC * 64, C], bf16, isOutput=False)
    pad = nc.declare_dram_parameter("pad", [1, T], bf16, isOutput=False)
    out = nc.declare_dram_parameter("out", [T, C], f32, isOutput=True)

    xT_r = xT.rearrange("(ko p) t -> p ko t", p=128)
    wqk_r = wqk.rearrange("(ko p) f -> p ko f", p=128)
    wv_r = wv.rearrange("(ko p) f -> p ko f", p=128)

    with tile.TileContext(nc) as tc:
        with (
            nc.allow_low_precision(
                reason="bf16 matmuls with fp32 PSUM accumulation"),
            tc.tile_pool(name="const", bufs=1) as cp,
            tc.tile_pool(name="qk", bufs=1) as qkp,
            tc.tile_pool(name="vp", bufs=1) as vp,
            tc.tile_pool(name="ctx", bufs=1) as ctxp,
        ):
            # ---- constants / weights ----
            wqk_sb = cp.tile([128, KO, HPC * 128], bf16, tag="wqk")
            wv_sb = cp.tile([128, KO, VW], bf16, tag="wv")
            bqk_sb = cp.tile([1, HPC * 128], bf16, tag="bqk")
            bv_sb = cp.tile([1, VW], bf16, tag="bv")
            wo_a = cp.tile([128, C], bf16, tag="wo_a")
            wo_b = cp.tile([64, C], bf16, tag="wo_b")
            pad_sb = cp.tile([1, T], bf16, tag="pad")
            ones_sb = cp.tile([128, 512], bf16, tag="ones")
            trin_sb = cp.tile([128, 128], bf16, tag="trin")
            iz_sb = cp.tile([128, 512], bf16, tag="iz")
            # denominator strip: head h's denom at partition 32h (matmul
            # base partitions must be 0/32/64); other partitions stay 1.0
            den_sb = cp.tile([65, 512], f32, tag="den")
            denr_sb = cp.tile([65, 512], bf16, tag="denr")

            # ---- persistent activations ----
            qt = [qkp.tile([64, T], bf16, tag=f"qt{h}", name=f"qt{h}")
                  for h in range(HPC)]
            kt = [qkp.tile([64, T], bf16, tag=f"kt{h}", name=f"kt{h}")
                  for h in range(HPC)]
            v_sb = vp.tile([128, NT, VW], bf16, tag="v")
            ctxa = ctxp.tile([128, T], bf16, tag="ctxa")
            ctxb = ctxp.tile([64, T], bf16, tag="ctxb")

            # ---- fused J loop: qkv(J) -> attention(J, all heads) -> Wo(J).
            # PSUM budget (8 banks): pp 1 + wo 1 + s 2x2 + o 2 = 8
            with (
                tc.tile_pool(name="xp", bufs=2) as xpool,
                tc.tile_pool(name="pp", bufs=1, space="PSUM") as pp,
                tc.tile_pool(name="wop", bufs=1, space="PSUM") as wop,
                tc.tile_pool(name="spool", bufs=2, space="PSUM") as spool,
                tc.tile_pool(name="opool", bufs=2, space="PSUM") as opool,
                tc.tile_pool(name="epool", bufs=5) as epool,
                tc.tile_pool(name="npool", bufs=2) as npool,
                tc.tile_pool(name="pout", bufs=3) as poutp,
            ):
                xp_tiles = {}

                def emit_xp_dma(Jn):
                    # one batched 3D DMA: measured 327 GB/s vs 200 for
                    # six per-k DMAs
                    xpt = xpool.tile([128, KO, 512], bf16, tag="x",
                                     name=f"xp{Jn}")
                    xp_tiles[Jn] = xpt
                    Js = Jn % NJ    # key NJ = next rep's block 0
                    nc.sync.dma_start(
                        xpt[:], xT_r[:, :, Js * 512:(Js + 1) * 512])

                def emit_qk_chain(Jn, h):
                    xpt = xp_tiles[Jn]
                    ps = pp.tile([128, 512], f32, tag="p1", name="psqk")
                    for k in range(KO):
                        nc.tensor.matmul(
                            ps[:], wqk_sb[:, k, h * 128:(h + 1) * 128],
                            xpt[:, k, :], start=(k == 0),
                            stop=(k == KO - 1 and not use_bqk))
                    if use_bqk:
                        nc.tensor.matmul(
                            ps[:], bqk_sb[0:1, h * 128:(h + 1) * 128],
                            ones_sb[0:1, :], start=False, stop=True)
                    nc.vector.tensor_copy(
                        qt[h][:, Jn * 512:(Jn + 1) * 512], ps[0:64, :])
                    nc.vector.tensor_copy(
                        kt[h][:, Jn * 512:(Jn + 1) * 512], ps[64:128, :])

                def emit_v_chain(Jn, sub):
                    xpt = xp_tiles[Jn]
                    pv = pp.tile([128, VW], f32, tag="p1", name="psv")
                    for k in range(KO):
                        nc.tensor.matmul(
                            pv[:], xpt[:, k, sub * 128:(sub + 1) * 128],
                            wv_sb[:, k, :], start=(k == 0), stop=False)
                    nc.tensor.matmul(pv[:], ones_sb[0:1, 0:128], bv_sb[0:1, :],
                                     start=False, stop=True)
                    nc.vector.tensor_copy(v_sb[:, Jn * 4 + sub, :], pv[:])

                def emit_wo(t):
                    # ctx for this block must be complete: force-emit any
                    # deferred normalize-part-2 for blocks <= t//4
                    while pending_n2 and pending_n2[0][0] <= t // 4:
                        pending_n2.pop(0)[1]()
                    po = poutp.tile([128, C], f32, tag="po")
                    for n in range(2):
                        wps = wop.tile([128, 384], f32, tag="wo", name="pswo")
                        nc.tensor.matmul(
                            wps[:], ctxa[:, t * 128:(t + 1) * 128],
                            wo_a[:, n * 384:(n + 1) * 384],
                            start=True, stop=False)
                        nc.tensor.matmul(
                            wps[:], ctxb[:, t * 128:(t + 1) * 128],
                            wo_b[:, n * 384:(n + 1) * 384],
                            start=False, stop=True)
                        nc.vector.tensor_copy(po[:, n * 384:(n + 1) * 384],
                                              wps[:])
                    # out stores ride the Pool DMA queue so x prefetches on
                    # the sync queue are never stuck behind them
                    nc.gpsimd.dma_start(out[t * 128:(t + 1) * 128, :], po[:])

                # timing harness: `reps` repeats the computation
                # back-to-back inside one NEFF
                tmps = []
                pending_n2 = []
                for _rep in range(reps):
                    if _rep == 0:
                        # loads ordered by first use: interleave W_qk and x
                        # per contraction subtile so chain k can start as
                        # soon as slice k lands
                        xpt0 = xpool.tile([128, KO, 512], bf16, tag="x",
                                          name="xp0")
                        xp_tiles[0] = xpt0
                        for k in range(KO):
                            nc.sync.dma_start(wqk_sb[:, k, :], wqk_r[:, k, :])
                            nc.sync.dma_start(xpt0[:, k, :],
                                              xT_r[:, k, 0:512])
                        for k in range(KO):
                            nc.gpsimd.dma_start(wv_sb[:, k, :], wv_r[:, k, :])
                        nc.gpsimd.dma_start(bqk_sb[:], bqk[:])
                        nc.gpsimd.dma_start(bv_sb[:], bv[:])
                        if use_pad:
                            nc.gpsimd.dma_start(pad_sb[:], pad[:])
                        nc.vector.memset(ones_sb[:], 1.0)
                        nc.vector.memset(den_sb[:], 1.0)
                        make_upper_triangular(nc, trin_sb[:], val=-3744.0,
                                              diag=False)
                        nc.gpsimd.memset(iz_sb[:, 128:512], 0.0)
                        make_identity(nc, iz_sb[:, 0:128])
                        nc.gpsimd.dma_start(wo_a[:], wo[0:128, :])
                        nc.gpsimd.dma_start(wo_b[:], wo[128:192, :])
                    else:
                        xp_tiles[0] = xp_tiles.pop(NJ)
                    emit_qk_chain(0, 0)
                    for sub in range(4):
                        emit_v_chain(0, sub)

                    for J in range(NJ):
                        # Filler work interleaved into this block's attention
                        # stream: next block's projection + previous block's
                        # Wo, fed to the PE while ACT works on exps.
                        # block 3's Wo rides the NEXT rep's J=0 stream so
                        # the rep tail never stalls on the last normalize
                        wo_sched = {0: [3] if _rep > 0 else [], 1: [0],
                                    2: [], 3: [1, 2]}
                        fillers = []
                        if J == 0:
                            fillers.append(lambda: emit_qk_chain(0, 1))
                            fillers.append(lambda: emit_qk_chain(0, 2))
                        if J + 1 < NJ:
                            fillers.append(lambda Jn=J + 1: emit_xp_dma(Jn))
                            for h in range(HPC):
                                fillers.append(
                                    lambda Jn=J + 1, hh=h: emit_qk_chain(Jn, hh))
                            for sub in range(4):
                                fillers.append(
                                    lambda Jn=J + 1, ss=sub: emit_v_chain(Jn, ss))
                        elif _rep + 1 < reps:
                            # prefetch next rep's first x block so the rep
                            # boundary has no DMA bubble
                            fillers.append(lambda: emit_xp_dma(NJ))
                        for Jw in wo_sched[J]:
                            for sub in range(4):
                                fillers.append(
                                    lambda tt=Jw * 4 + sub: emit_wo(tt))

                        nrows = 4 * J + 4
                        npairs = nrows // 2
                        ticks = HPC * npairs
                        stride = max(1, ticks // max(1, len(fillers)))
                        tick = 0

                        def do_av(item, h, nrows):
                            eTq, rowsq = item
                            for idx, tkr in enumerate(rowsq):
                                off = max(0, (tkr - 4 * J) * 128)
                                # compact packing: row0 at [off0:512],
                                # row1 at [512:1024-off1]
                                c0 = off if idx == 0 else 512
                                c1 = 512 if idx == 0 else 1024 - off
                                nc.tensor.matmul(
                                    o_ps[:, off:512],
                                    v_sb[:, tkr, h * HS:h * HS + 65],
                                    eTq[:, c0:c1],
                                    start=(tkr == 0), stop=(tkr == nrows - 1))

                        for h in range(HPC):
                            o_ps = opool.tile([65, 512], f32, tag="o")
                            av_q = []
                            for pr in range(npairs):
                                rows = (2 * pr, 2 * pr + 1)
                                diag = rows[0] >= 4 * J
                                off0 = max(0, (rows[0] - 4 * J) * 128)
                                off1 = max(0, (rows[1] - 4 * J) * 128)
                                s_ps = spool.tile([128, 1024], f32, tag="s")
                                eT = epool.tile([128, 1024], bf16, tag="e")
                                for idx, tkr in enumerate(rows):
                                    off = off0 if idx == 0 else off1
                                    n0 = off if idx == 0 else 512
                                    n1 = 512 if idx == 0 else 1024 - off
                                    last = not use_pad and not diag
                                    nc.tensor.matmul(
                                        s_ps[:, n0:n1],
                                        kt[h][:, tkr * 128:(tkr + 1) * 128],
                                        qt[h][:, J * 512 + off:(J + 1) * 512],
                                        start=True, stop=last)
                                    if use_pad:
                                        nc.tensor.matmul(
                                            s_ps[:, n0:n1],
                                            pad_sb[0:1, tkr * 128:(tkr + 1) * 128],
                                            ones_sb[0:1, 0:512 - off],
                                            start=False, stop=not diag)
                                    if diag:
                                        # += strict_upper(-3744) on the
                                        # diagonal block (first 128 cols of
                                        # the packed span): tri_neg^T @ I
                                        nc.tensor.matmul(
                                            s_ps[:, n0:n0 + 128],
                                            trin_sb[:],
                                            iz_sb[:, 0:128],
                                            start=False, stop=True)
                                # one exp over the exact packed valid span;
                                # masked entries underflow to exactly 0
                                nc.scalar.activation(
                                    eT[:, off0:1024 - off1],
                                    s_ps[:, off0:1024 - off1], Exp,
                                    scale=1.0)
                                av_q.append((eT, rows))
                                if len(av_q) > 3:
                                    do_av(av_q.pop(0), h, nrows)
                                tick += 1
                                if tick >= 4 and pending_n2 and \
                                        pending_n2[0][0] < J:
                                    pending_n2.pop(0)[1]()
                                if tick % stride == 0 and fillers:
                                    fillers.pop(0)()
                            for item in av_q:
                                do_av(item, h, nrows)
                            # normalize part 1 (immediate, releases o_ps
                            # fast): stash the denom row in the per-J strip
                            # and evacuate the o rows
                            nc.vector.tensor_copy(
                                den_sb[32 * h:32 * h + 1, :], o_ps[64:65, :])
                            tmp = npool.tile([64, 512], f32, tag=f"tmp{h}")
                            nc.vector.tensor_copy(tmp[:], o_ps[0:64, :])
                            tmps.append(tmp)

                        # normalize part 2, once per J: ONE reciprocal over
                        # the [3,512] strip (costs the same 3.4us as
                        # [1,512]); the per-head bf16 broadcast matmul +
                        # multiply into ctx are DEFERRED into the next
                        # block's stream so the PE queue never stalls on
                        # the reciprocal
                        nc.vector.reciprocal(denr_sb[:], den_sb[:])

                        def n2(h, J, tmp):
                            bc = spool.tile([64, 512], f32, tag="s",
                                            name="bc")
                            nc.tensor.matmul(
                                bc[:], ones_sb[32 * h:32 * h + 1, 0:64],
                                denr_sb[32 * h:32 * h + 1, :], start=True,
                                stop=True)
                            dst = (ctxa[64 * h:64 * h + 64,
                                        J * 512:(J + 1) * 512]
                                   if h < 2 else ctxb[:, J * 512:(J + 1) * 512])
                            nc.vector.tensor_tensor(
                                dst, tmp[:], bc[:], mybir.AluOpType.mult)
                        for h in range(HPC):
                            pending_n2.append(
                                (J, lambda h=h, J=J, tmp=tmps[h]: n2(h, J, tmp)))
                        tmps = []

                        for f in fillers:  # flush leftovers
                            f()
                    if _rep == reps - 1:
                        for _, f in pending_n2:
                            f()
                        pending_n2 = []
                        for sub in range(4):
                            emit_wo((NJ - 1) * 4 + sub)

    nc.finalize()
    return nc


def _get_nc(use_pad: bool, use_bqk: bool, reps: int = 1):
    key = (use_pad, use_bqk, reps)
    if key not in _NC_CACHE:
        _NC_CACHE[key] = build_nc(use_pad, use_bqk, reps)
    return _NC_CACHE[key]


def _core_inputs(c, x, attention_mask, Wqkv_w, Wqkv_b, Wo_w, use_pad):
    b, g = c // 4, c % 4
    rev = g >= 2
    heads = [3 * g + i for i in range(HPC)]

    xb = x[b]
    if rev:
        xb = xb[::-1, :]
    xT = np.ascontiguousarray(xb.T).astype(bft)

    wqk = np.empty((HPC * 128, C), dtype=np.float32)
    bqk = np.empty((1, HPC * 128), dtype=np.float32)
    wv = np.zeros((VW, C), dtype=np.float32)   # pad rows stay zero
    bv = np.zeros((1, VW), dtype=np.float32)
    wo = np.empty((HPC * 64, C), dtype=np.float32)
    for i, hd in enumerate(heads):
        qs, ks, vs = hd * 64, C + hd * 64, 2 * C + hd * 64
        # 1/8 softmax scale folded into q (exact in bf16: power of two)
        wqk[i * 128:i * 128 + 64] = Wqkv_w[qs:qs + 64] * 0.125
        wqk[i * 128 + 64:(i + 1) * 128] = Wqkv_w[ks:ks + 64]
        bqk[0, i * 128:i * 128 + 64] = Wqkv_b[qs:qs + 64] * 0.125
        bqk[0, i * 128 + 64:(i + 1) * 128] = Wqkv_b[ks:ks + 64]
        wv[i * HS:i * HS + 64] = Wqkv_w[vs:vs + 64]
        bv[0, i * HS:i * HS + 64] = Wqkv_b[vs:vs + 64]
        bv[0, i * HS + 64] = 1.0
        wo[i * 64:(i + 1) * 64] = Wo_w[:, hd * 64:(hd + 1) * 64].T

    if use_pad:
        padv = ((1.0 - attention_mask[b].astype(np.float32)) * -3744.0)
        if rev:
            padv = padv[::-1]
        padv = np.ascontiguousarray(padv.reshape(1, T))
    else:
        padv = np.zeros((1, T), dtype=np.float32)

    return {
        "xT": xT,
        "wqk": np.ascontiguousarray(wqk.T).astype(bft),
        "wv": np.ascontiguousarray(wv.T).astype(bft),
        "bqk": bqk.astype(bft),
        "bv": bv.astype(bft),
        "wo": np.ascontiguousarray(wo).astype(bft),
        "pad": padv.astype(bft),
    }


def run_cores(x, attention_mask, Wqkv_w, Wqkv_b, Wo_w, trace=False):
    use_pad = not bool(np.all(attention_mask == 1))
    use_bqk = bool(np.any(Wqkv_b[:2 * C] != 0.0))
    nc = _get_nc(use_pad, use_bqk)
    in_maps = [
        _core_inputs(c, x, attention_mask, Wqkv_w, Wqkv_b, Wo_w, use_pad)
        for c in range(NCORES)
    ]
    return run_bass_kernel_spmd(nc, in_maps, list(range(NCORES)), trace=trace)


def kernel(x, attention_mask, Wqkv_w, Wqkv_b, Wo_w, Wo_b):
    x = np.asarray(x, dtype=np.float32)
    attention_mask = np.asarray(attention_mask)
    Wqkv_w = np.asarray(Wqkv_w, dtype=np.float32)
    Wqkv_b = np.asarray(Wqkv_b, dtype=np.float32)
    Wo_w = np.asarray(Wo_w, dtype=np.float32)
    Wo_b = np.asarray(Wo_b, dtype=np.float32)

    res = run_cores(x, attention_mask, Wqkv_w, Wqkv_b, Wo_w)
    out = np.zeros((B, T, C), dtype=np.float32)
    for c in range(NCORES):
        b, g = c // 4, c % 4
        po = res.results[c]["out"]
        if g >= 2:
            po = po[::-1, :]
        out[b] += po
    out += Wo_b
    return out.astype(np.float32)
